# revision 21
# baseline (speedup 1.0000x reference)
"""NRI-style GNN encoder (gnn_message_passing) on 8 Trainium2 NeuronCores.

Data-parallel over batch: core b computes batch element b end-to-end.

v2 design (vs the spill-based v1):
  - Edge set padded to all N*N=16384 (r,s) pairs -> 32 aligned macros of 512;
    self-pairs are computed wastefully (0.8%) and dropped on the host. The
    aggregation subtracts the self-pair column (stride-129 AP).
  - 2-op ELU: elu(y)+1 == max(y+b+1, min(exp(y+b), 1)) exactly, so each edge
    activation is one ACT Exp + one clip + one scalar_tensor_tensor
    (z = (psum + c) max tclip), with b+1 pre-folded into the gather
    stationaries for the pre-layers.
  - Gathers use fp8 DoubleRow matmuls with hi+lo residual splitting:
    u ~ fp8(u) + fp8(u - fp8(u)) (~0.1% error), and the one-hot rel patterns
    are exact in fp8. K=256 per instruction at 0.5 cyc/row halves gather PE
    cost. The send pattern is a tiled identity (tiny const, no big DMA).
  - Aggregation via segmented tensor_reduce (edges of one receiver are a
    contiguous 128-block after padding): no PE transposes, no psum copies.
  - z1 (skip input) stays resident in SBUF [128,2,16384] fp16: no DRAM spill.
  - Pass-1 stage A optionally computes t = exp(u+b)*exp(v) factorized on DVE
    (per-receiver column scalars) instead of ACT Exp, to balance ACT vs DVE.
  - Output computed as out^T [2, E'] on PE (feat-major z4); bias and
    un-padding on the host.
"""

import os
import sys

for _p in ("/opt/trn_rl_repo",):
    if _p not in sys.path:
        sys.path.insert(0, _p)

import numpy as np
import ml_dtypes

import concourse.bass as bass
import concourse.tile as tile
from concourse import bacc, mybir
from concourse.bass_utils import run_bass_kernel_spmd

DT = mybir.dt
AF = mybir.ActivationFunctionType
ALU = mybir.AluOpType
DRMODE = mybir.MatmulPerfMode.DoubleRow

B, N, T, D, H, NE = 8, 128, 49, 4, 256, 2
E = N * (N - 1)          # 16256 real edges
EP = N * N               # 16384 padded edges
F = T * D                # 196
MACRO = 512
NMAC = EP // MACRO       # 32

# ---------------- tuning knobs (per-macro engine assignment) ----------------
# NOTE: GPSIMD (Pool) cannot access PSUM -> psum-reading STTs must be DVE;
# Pool only gets SBUF-only ops (tclips, agg reduce).
P1A_FACT = [False] * NMAC                      # True: DVE-factorized exp
STT_1A = ['p' if i % 2 == 1 else 'v' for i in range(NMAC)]
STT_1B = ['v'] * NMAC
STT_2A = ['p' if i % 4 == 1 else 'v' for i in range(NMAC)]
STT_2B = ['v'] * NMAC
TCLIP_1A = ['g'] * NMAC                        # sbuf op: 'v' DVE / 'g' Pool
TCLIP_1B = ['g'] * NMAC
TCLIP_2A = ['g'] * NMAC
TCLIP_2B = ['g'] * NMAC
OUT_COPY = ['a'] * NMAC
AGG_ENG = ['v'] * NMAC   # gpsimd tensor_reduce can't do free-axis reduce


def _mk_layout(entries):
    out, c = {}, 0
    for name, w in entries:
        out[name] = (c, w)
        c += w
    return out, c

PK32, C32 = _mk_layout([
    ("wn1a", 256), ("wn1b", 256), ("wn1l2", 512),
    ("a1s", 512), ("b1s", 512),
    ("wn2l1", 512), ("wn2l2", 512), ("a2s", 512), ("b2s", 512),
    ("bcols", 32), ("onesrow", 128), ("berows", 512),
])
PK16, C16 = _mk_layout([
    ("we1l2", 512), ("c2s", 512), ("we2l2", 512), ("ows", 4),
])

# bias columns: 2 consecutive cols per key (fh0, fh1) except neg1
BC = dict(neg1=0,
          be2=1, be21=3, be4=5, be41=7,
          nb11=9, nb111=11, nb12=13, nb121=15,
          nb21=17, nb211=19, nb22=21, nb221=23,
          be1=25)

_PROG_CACHE = {}
LAST_EXEC_NS = None


def _build_program():
    nc = bacc.Bacc(
        "TRN2",
        target_bir_lowering=False,
        debug=False,
        enable_asserts=True,
        num_devices=8,
    )

    f32, f16, f8 = DT.float32, DT.float16, DT.float8e4

    def din(name, shape, dt=f32):
        return nc.dram_tensor(name, list(shape), dt, kind="ExternalInput").ap()

    xt_in = din("xt", [128, 2, 128])
    pk32 = din("pk32", [128, C32], f32)
    pk16 = din("pk16", [128, C16], f16)
    rec16 = din("rec16", [128, EP], f16)
    send16 = din("send16", [128, MACRO], f16)

    out_d = nc.dram_tensor("outT", [2, EP], f32, kind="ExternalOutput").ap()

    offs = [m * MACRO for m in range(NMAC)]

    with tile.TileContext(nc) as tc:
        with (
            tc.tile_pool(name="const", bufs=1) as cpool,
            tc.tile_pool(name="rel", bufs=1) as relpool,
            tc.tile_pool(name="z1p", bufs=1) as z1pool,
            tc.tile_pool(name="wk", bufs=3) as wk,
            tc.tile_pool(name="ps_a", bufs=2, space="PSUM") as ps_a,
            tc.tile_pool(name="ps_b", bufs=4, space="PSUM") as ps_b,
        ):
            # ---------------- loads ----------------
            def ctile(ap_dram, shape, dt=f32, name="c"):
                t = cpool.tile(shape, dt, name=name)
                nc.sync.dma_start(t[:], ap_dram)
                return t

            xt = ctile(xt_in, [128, 2, 128], name="xt")
            p32 = ctile(pk32, [128, C32], f32, name="p32")
            p16 = ctile(pk16, [128, C16], f16, name="p16")
            send_sb = relpool.tile([128, MACRO], f16, name="send_sb")
            nc.sync.dma_start(send_sb[:], send16)
            rec_sb = relpool.tile([128, EP], f16, name="rec_sb")
            for c0 in range(0, EP, 4096):
                nc.sync.dma_start(rec_sb[:, c0:c0 + 4096],
                                  rec16[:, c0:c0 + 4096])

            def c32(name, hview=False):
                c0, w = PK32[name]
                ap = p32[:, c0:c0 + w]
                if hview:
                    ap = ap.rearrange("p (h o) -> p h o", h=2)
                return ap

            def c16(name, hview=False):
                c0, w = PK16[name]
                ap = p16[:, c0:c0 + w]
                if hview:
                    ap = ap.rearrange("p (h o) -> p h o", h=2)
                return ap

            bc0 = PK32["bcols"][0]

            def bcol(key, fh=0):
                c = bc0 + BC[key] + fh
                return p32[:, c:c + 1]

            wn1a = c32("wn1a")
            wn1b = c32("wn1b")[0:68, :]
            wn1l2 = c32("wn1l2", hview=True)
            a1s = c32("a1s", hview=True)
            b1s = c32("b1s", hview=True)
            wn2l1 = c32("wn2l1", hview=True)
            wn2l2 = c32("wn2l2", hview=True)
            a2s = c32("a2s", hview=True)
            b2s = c32("b2s", hview=True)
            onesrow = c32("onesrow")[0:1, :]          # [1, 128] of 1.0
            berows = c32("berows")                    # [128, 512]; row 0 used
            we1l2 = c16("we1l2", hview=True)
            c2s = c16("c2s", hview=True)
            we2l2 = c16("we2l2", hview=True)
            ows = c16("ows", hview=True)              # [128, 2, 2]

            z1 = z1pool.tile([128, 2, EP], f16, name="z1")

            # ---------------- node helpers ----------------
            def node_mm(lhsT_h, rhs):
                ps = ps_a.tile([128, 2, MACRO], f32, name="psA", tag="psA")
                for kh in range(2):
                    for oh in range(2):
                        nc.tensor.matmul(
                            ps[:, oh, :128],
                            lhsT_h[:, kh, oh * 128:(oh + 1) * 128],
                            rhs[:, kh, :],
                            start=(kh == 0), stop=(kh == 1))
                return ps

            PLUS1 = {"nb11": "nb111", "nb12": "nb121",
                     "nb21": "nb211", "nb22": "nb221"}

            def elu_node(ps, bkey, out_name):
                z = cpool.tile([128, 2, 128], f32, name=out_name)
                t = wk.tile([128, 2, 128], f32, name="tn", tag="tn", bufs=2)
                for fh in range(2):
                    nc.scalar.activation(t[:, fh, :], ps[:, fh, :128],
                                         AF.Exp, bias=bcol(bkey, fh))
                    nc.vector.tensor_scalar(t[:, fh, :], t[:, fh, :],
                                            1.0, 0.0, ALU.min, ALU.max)
                    nc.vector.scalar_tensor_tensor(
                        z[:, fh, :], ps[:, fh, :128],
                        bcol(PLUS1[bkey], fh),
                        t[:, fh, :], ALU.add, ALU.max)
                return z

            # ---------------- node stage 1 ----------------
            ps1 = ps_a.tile([128, 2, MACRO], f32, name="psA", tag="psA")
            for oh in range(2):
                nc.tensor.matmul(ps1[:, oh, :128],
                                 wn1a[:, oh * 128:(oh + 1) * 128],
                                 xt[:, 0, :], start=True, stop=False)
                nc.tensor.matmul(ps1[:, oh, :128],
                                 wn1b[:, oh * 128:(oh + 1) * 128],
                                 xt[0:68, 1, :], start=False, stop=True)
            zn1 = elu_node(ps1, "nb11", "zn1")

            ps2 = node_mm(wn1l2, zn1)
            zh1 = elu_node(ps2, "nb12", "zh1")   # feat-major [if-half, 2, n]

            # u1/v1 feat-major (for EU/EV columns)
            def uv_feat(ws, name):
                ps = node_mm(ws, zh1)
                u = cpool.tile([128, 2, 128], f32, name=name)
                nc.vector.tensor_copy(u[:], ps[:, :, 0:128])
                return u

            u1f = uv_feat(a1s, "u1f")
            v1f = uv_feat(b1s, "v1f")

            eu1 = cpool.tile([128, 2, 128], f32, name="eu1")
            ev1 = cpool.tile([128, 2, 128], f16, name="ev1")
            for fh in range(2):
                nc.scalar.activation(eu1[:, fh, :], u1f[:, fh, :], AF.Exp,
                                     bias=bcol("be1", fh))
                nc.scalar.activation(ev1[:, fh, :], v1f[:, fh, :], AF.Exp)

            # node-major fp8 hi/lo DR stationaries [n, 2(hi/lo), 256]
            def uv_node(zsrc, ws, fold_row, name):
                ps = ps_b.tile([128, MACRO], f32, name="psB", tag="psB")
                last_is_fold = fold_row is not None
                for kh in range(2):
                    nc.tensor.matmul(
                        ps[:, :256], zsrc[:, kh, :], ws[:, kh, :],
                        start=(kh == 0),
                        stop=(kh == 1 and not last_is_fold))
                if last_is_fold:
                    nc.tensor.matmul(ps[:, :256], onesrow, fold_row,
                                     start=False, stop=True)
                uv = cpool.tile([128, 256], f16, name=name)
                nc.vector.tensor_copy(uv[:], ps[:, :256])
                return uv

            uv1u = uv_node(zh1, a1s, berows[0:1, 0:256], "uv1u")  # +be1+1
            uv1v = uv_node(zh1, b1s, None, "uv1v")

            # ---------------- pass 1 ----------------
            def p1_stageA(off, mi):
                ps = ps_a.tile([128, 2, MACRO], f32, name="psA", tag="psA")
                for fh in range(2):
                    nc.tensor.matmul(
                        ps[:, fh, :], uv1u[:, fh * 128:(fh + 1) * 128],
                        rec_sb[:, off:off + MACRO],
                        start=True, stop=False)
                    nc.tensor.matmul(
                        ps[:, fh, :], uv1v[:, fh * 128:(fh + 1) * 128],
                        send_sb[:], start=False, stop=True)
                t = wk.tile([128, 2, MACRO], f16, name="t1a", tag="t1a",
                            bufs=4)
                if P1A_FACT[mi]:
                    for blk in range(4):
                        r = (off // 128) + blk
                        for fh in range(2):
                            nc.vector.tensor_scalar(
                                t[:, fh, blk * 128:(blk + 1) * 128],
                                ev1[:, fh, :], eu1[:, fh, r:r + 1], 0.0,
                                ALU.mult, ALU.max)
                else:
                    nc.scalar.activation(
                        t[:].rearrange("p a b -> p (a b)"),
                        ps[:].rearrange("p a b -> p (a b)"),
                        AF.Exp, bias=bcol("neg1"))
                z1a = wk.tile([128, 2, MACRO], f16, name="z1a", tag="z1a",
                              bufs=4)
                if STT_1A[mi] == 'v':
                    nc.vector.scalar_tensor_tensor(
                        z1a[:].rearrange("p a b -> p (a b)"),
                        t[:].rearrange("p a b -> p (a b)"), 1.0,
                        ps[:].rearrange("p a b -> p (a b)"), ALU.min, ALU.max)
                else:
                    yc = wk.tile([128, 2, MACRO], f16, name="yc1a",
                                 tag="yc1a", bufs=3)
                    nc.scalar.copy(yc[:].rearrange("p a b -> p (a b)"),
                                   ps[:].rearrange("p a b -> p (a b)"))
                    nc.gpsimd.tensor_scalar(
                        t[:].rearrange("p a b -> p (a b)"),
                        t[:].rearrange("p a b -> p (a b)"),
                        1.0, 0.0, ALU.min, ALU.max)
                    nc.vector.tensor_tensor(
                        z1a[:].rearrange("p a b -> p (a b)"),
                        yc[:].rearrange("p a b -> p (a b)"),
                        t[:].rearrange("p a b -> p (a b)"), ALU.max)
                return z1a

            def p1_stageB(off, z1a, mi):
                t = wk.tile([128, 2, MACRO], f16, name="t1b", tag="t1b",
                            bufs=4)
                ceng = nc.vector if TCLIP_1B[mi] == 'v' else nc.gpsimd
                for oh in range(2):
                    ps = ps_b.tile([128, MACRO], f32, name="psB", tag="psB")
                    for fh in range(2):
                        nc.tensor.matmul(
                            ps[:],
                            we1l2[:, fh, oh * 128:(oh + 1) * 128],
                            z1a[:, fh, :], start=(fh == 0), stop=(fh == 1))
                    nc.scalar.activation(t[:, oh, :], ps[:], AF.Exp,
                                         bias=bcol("be2", oh))
                    ceng.tensor_scalar(t[:, oh, :], t[:, oh, :],
                                       1.0, 0.0, ALU.min, ALU.max)
                    nc.vector.scalar_tensor_tensor(
                        z1[:, oh, off:off + MACRO], ps[:],
                        bcol("be21", oh), t[:, oh, :], ALU.add, ALU.max)

            def p1_stageC(aggf, off, mi):
                b0 = off // 128
                part = wk.tile([128, 2, 4, 8], f16, name="aggp", tag="aggp",
                               bufs=2)
                with nc.allow_low_precision(
                        reason="16-sum fp16 partials; combined in fp32"):
                    nc.vector.tensor_reduce(
                        part[:],
                        z1[:, :, off:off + MACRO].rearrange(
                            "p h (b s c) -> p h b s c", b=4, c=16),
                        mybir.AxisListType.X, ALU.add)
                nc.vector.tensor_reduce(
                    aggf[:, :, b0:b0 + 4], part[:],
                    mybir.AxisListType.X, ALU.add)

            aggf = cpool.tile([128, 2, 128], f32, name="aggf")
            recs = []
            for mi, off in enumerate(offs):
                z1a = p1_stageA(off, mi)
                recs.append((off, z1a, mi))
                if len(recs) >= 2:
                    o, za, m = recs[-2]
                    p1_stageB(o, za, m)
                if len(recs) >= 3:
                    o, _, m = recs[-3]
                    p1_stageC(aggf, o, m)
            o, za, m = recs[-1]
            p1_stageB(o, za, m)
            for o, _, m in recs[-2:]:
                p1_stageC(aggf, o, m)

            # subtract self-pair columns z1[:, :, 129*n]
            dcols = z1[:, :, 0:EP:129]
            agg2 = cpool.tile([128, 2, 128], f32, name="agg2")
            nc.vector.tensor_tensor(agg2[:], aggf[:], dcols, ALU.subtract)

            # ---------------- node stage 2 ----------------
            ps3 = node_mm(wn2l1, agg2)
            zn2 = elu_node(ps3, "nb21", "zn2")
            ps4 = node_mm(wn2l2, zn2)
            zh2 = elu_node(ps4, "nb22", "zh2")

            uv2u = uv_node(zh2, a2s, berows[0:1, 256:512], "uv2u")  # +be3+1
            uv2v = uv_node(zh2, b2s, None, "uv2v")

            # ---------------- pass 2 ----------------
            def p2_stageA(off, mi):
                ps = ps_a.tile([128, 2, MACRO], f32, name="psA", tag="psA")
                for oh in range(2):
                    for fh in range(2):
                        nc.tensor.matmul(
                            ps[:, oh, :],
                            c2s[:, fh, oh * 128:(oh + 1) * 128],
                            z1[:, fh, off:off + MACRO],
                            start=(fh == 0), stop=False)
                for fh in range(2):
                    nc.tensor.matmul(
                        ps[:, fh, :], uv2u[:, fh * 128:(fh + 1) * 128],
                        rec_sb[:, off:off + MACRO],
                        start=False, stop=False)
                    nc.tensor.matmul(
                        ps[:, fh, :], uv2v[:, fh * 128:(fh + 1) * 128],
                        send_sb[:], start=False, stop=True)
                t = wk.tile([128, 2, MACRO], f16, name="t2a", tag="t2a",
                            bufs=4)
                nc.scalar.activation(
                    t[:].rearrange("p a b -> p (a b)"),
                    ps[:].rearrange("p a b -> p (a b)"),
                    AF.Exp, bias=bcol("neg1"))
                z2a = wk.tile([128, 2, MACRO], f16, name="z2a", tag="z2a",
                              bufs=4)
                if STT_2A[mi] == 'v':
                    nc.vector.scalar_tensor_tensor(
                        z2a[:].rearrange("p a b -> p (a b)"),
                        t[:].rearrange("p a b -> p (a b)"), 1.0,
                        ps[:].rearrange("p a b -> p (a b)"), ALU.min, ALU.max)
                else:
                    yc = wk.tile([128, 2, MACRO], f16, name="yc2a",
                                 tag="yc2a", bufs=3)
                    nc.scalar.copy(yc[:].rearrange("p a b -> p (a b)"),
                                   ps[:].rearrange("p a b -> p (a b)"))
                    nc.gpsimd.tensor_scalar(
                        t[:].rearrange("p a b -> p (a b)"),
                        t[:].rearrange("p a b -> p (a b)"),
                        1.0, 0.0, ALU.min, ALU.max)
                    nc.vector.tensor_tensor(
                        z2a[:].rearrange("p a b -> p (a b)"),
                        yc[:].rearrange("p a b -> p (a b)"),
                        t[:].rearrange("p a b -> p (a b)"), ALU.max)
                return z2a

            def p2_stageB(off, z2a, mi):
                t = wk.tile([128, 2, MACRO], f16, name="t2b", tag="t2b",
                            bufs=4)
                z4 = wk.tile([128, 2, MACRO], f16, name="z4", tag="z4",
                             bufs=3)
                ceng = nc.vector if TCLIP_2B[mi] == 'v' else nc.gpsimd
                for oh in range(2):
                    ps = ps_b.tile([128, MACRO], f32, name="psB", tag="psB")
                    for fh in range(2):
                        nc.tensor.matmul(
                            ps[:],
                            we2l2[:, fh, oh * 128:(oh + 1) * 128],
                            z2a[:, fh, :], start=(fh == 0), stop=(fh == 1))
                    nc.scalar.activation(t[:, oh, :], ps[:], AF.Exp,
                                         bias=bcol("be4", oh))
                    ceng.tensor_scalar(t[:, oh, :], t[:, oh, :],
                                       1.0, 0.0, ALU.min, ALU.max)
                    nc.vector.scalar_tensor_tensor(
                        z4[:, oh, :], ps[:],
                        bcol("be41", oh), t[:, oh, :], ALU.add, ALU.max)
                return z4

            def p2_stageO(off, z4, mi):
                opt = ps_b.tile([128, MACRO], f32, name="psB", tag="psB")
                op = opt[0:2, :]
                for fh in range(2):
                    nc.tensor.matmul(op, ows[:, fh, :], z4[:, fh, :],
                                     start=(fh == 0), stop=(fh == 1))
                osb = wk.tile([2, MACRO], f32, name="osb", tag="osb", bufs=2)
                if OUT_COPY[mi] == 'a':
                    nc.scalar.copy(osb[:], op)
                elif OUT_COPY[mi] == 'v':
                    nc.vector.tensor_copy(osb[:], op)
                else:
                    nc.gpsimd.tensor_copy(osb[:], op)
                nc.sync.dma_start(out_d[:, off:off + MACRO], osb[:])

            recs2 = []
            for mi, off in enumerate(offs):
                z2a = p2_stageA(off, mi)
                recs2.append((off, z2a, mi))
                if len(recs2) >= 2:
                    o, za, m = recs2[-2]
                    recs2[-2] = (o, p2_stageB(o, za, m), m)
                if len(recs2) >= 3:
                    o, z4, m = recs2[-3]
                    p2_stageO(o, z4, m)
            o, za, m = recs2[-1]
            recs2[-1] = (o, p2_stageB(o, za, m), m)
            for o, z4, m in recs2[-2:]:
                p2_stageO(o, z4, m)

    nc.compile()
    return nc


def _prep_inputs(inputs):
    f = lambda a: np.ascontiguousarray(np.asarray(a, dtype=np.float32))
    cs = lambda w: w.sum(axis=0)

    n1w1, n1b1 = f(inputs["n1w1"]), f(inputs["n1b1"])
    n1w2, n1b2 = f(inputs["n1w2"]), f(inputs["n1b2"])
    e1w1, e1b1 = f(inputs["e1w1"]), f(inputs["e1b1"])
    e1w2, e1b2 = f(inputs["e1w2"]), f(inputs["e1b2"])
    n2w1, n2b1 = f(inputs["n2w1"]), f(inputs["n2b1"])
    n2w2, n2b2 = f(inputs["n2w2"]), f(inputs["n2b2"])
    e2w1, e2b1 = f(inputs["e2w1"]), f(inputs["e2b1"])
    e2w2, e2b2 = f(inputs["e2w2"]), f(inputs["e2b2"])
    ow, ob = f(inputs["ow"]), f(inputs["ob"])

    A1, B1 = e1w1[:H], e1w1[H:]
    A2, B2, C2 = e2w1[:H], e2w1[H:2 * H], e2w1[2 * H:]

    e1w2_h = e1w2.astype(np.float16)
    C2_h = C2.astype(np.float16)
    e2w2_h = e2w2.astype(np.float16)
    ow_h = ow.astype(np.float16)

    # inputs to edge layers are stored as z = elu(.)+1, so fold -colsum(W)
    be1 = e1b1 - cs(A1) - cs(B1)
    be2 = e1b2 - cs(e1w2_h.astype(np.float32))
    be3 = e2b1 - cs(A2) - cs(B2) - cs(C2_h.astype(np.float32))
    be4 = e2b2 - cs(e2w2_h.astype(np.float32))
    nb11 = n1b1
    nb12 = n1b2 - cs(n1w2)
    nb21 = n2b1 - 127.0 * cs(n2w1)
    nb22 = n2b2 - cs(n2w2)
    ob_adj = ob - cs(ow_h.astype(np.float32))

    def sqh(w):
        return np.ascontiguousarray(
            w.reshape(2, 128, -1).transpose(1, 0, 2).reshape(128, -1))

    pk32 = np.zeros((128, C32), np.float32)

    def put32(name, arr):
        c0, w = PK32[name]
        pk32[:arr.shape[0], c0:c0 + w] = arr

    put32("wn1a", n1w1[:128])
    put32("wn1b", n1w1[128:])
    put32("wn1l2", sqh(n1w2))
    put32("a1s", sqh(A1))
    put32("b1s", sqh(B1))
    put32("wn2l1", sqh(n2w1))
    put32("wn2l2", sqh(n2w2))
    put32("a2s", sqh(A2))
    put32("b2s", sqh(B2))

    bc0 = PK32["bcols"][0]

    def putcol(idx, val):
        pk32[:, bc0 + idx] = val

    putcol(BC["neg1"], -1.0)
    for key, v in [("be2", be2), ("be21", be2 + 1.0),
                   ("be4", be4), ("be41", be4 + 1.0),
                   ("nb11", nb11), ("nb111", nb11 + 1.0),
                   ("nb12", nb12), ("nb121", nb12 + 1.0),
                   ("nb21", nb21), ("nb211", nb21 + 1.0),
                   ("nb22", nb22), ("nb221", nb22 + 1.0),
                   ("be1", be1)]:
        vv = v.reshape(2, 128)
        putcol(BC[key], vv[0])
        putcol(BC[key] + 1, vv[1])

    c0, w = PK32["onesrow"]
    pk32[0, c0:c0 + w] = 1.0
    c0, w = PK32["berows"]
    pk32[0, c0:c0 + 256] = be1 + 1.0
    pk32[0, c0 + 256:c0 + 512] = be3 + 1.0

    pk16 = np.zeros((128, C16), np.float16)

    def put16(name, arr):
        c0, w = PK16[name]
        pk16[:arr.shape[0], c0:c0 + w] = arr

    put16("we1l2", sqh(e1w2_h.astype(np.float32)).astype(np.float16))
    put16("c2s", sqh(C2_h.astype(np.float32)).astype(np.float16))
    put16("we2l2", sqh(e2w2_h.astype(np.float32)).astype(np.float16))
    put16("ows", sqh(ow_h.astype(np.float32)).astype(np.float16))

    recT = np.zeros((128, EP), np.float16)
    for r in range(N):
        recT[r, r * 128:(r + 1) * 128] = 1.0
    send = np.zeros((128, MACRO), np.float16)
    for blk in range(4):
        for s in range(128):
            send[s, blk * 128 + s] = 1.0

    shared = dict(pk32=pk32, pk16=pk16,
                  rec16=np.ascontiguousarray(recT),
                  send16=np.ascontiguousarray(send))
    return shared, ob_adj


def kernel(**inputs):
    global LAST_EXEC_NS
    if "prog" not in _PROG_CACHE:
        _PROG_CACHE["prog"] = _build_program()
    nc = _PROG_CACHE["prog"]

    shared, ob_adj = _prep_inputs(inputs)
    x = np.asarray(inputs["x"], dtype=np.float32)
    in_maps = []
    for b in range(B):
        m = dict(shared)
        xb = x[b].reshape(N, F)
        xtb = np.zeros((128, 2, 128), np.float32)
        xtb[:, 0, :] = xb[:, 0:128].T
        xtb[0:68, 1, :] = xb[:, 128:196].T
        m["xt"] = np.ascontiguousarray(xtb)
        in_maps.append(m)

    trace = os.environ.get("KERNEL_TRACE", "0") == "1"
    try:
        res = run_bass_kernel_spmd(nc, in_maps, core_ids=list(range(8)),
                                   trace=trace)
    except ModuleNotFoundError:
        res = run_bass_kernel_spmd(nc, in_maps, core_ids=list(range(8)),
                                   trace=False)
    if trace and res.exec_time_ns is not None:
        LAST_EXEC_NS = res.exec_time_ns
        print(f"HW exec time: {res.exec_time_ns} ns")

    off_mask = (~np.eye(N, dtype=bool)).reshape(-1)
    outs = []
    for b in range(B):
        oT = np.asarray(res.results[b]["outT"], np.float32)
        outs.append(oT.T[off_mask] + ob_adj[None, :])
    return np.stack(outs, axis=0).astype(np.float32)
